# revision 55
# baseline (speedup 1.0000x reference)
"""Trainium2 Bass kernel v6 for nn_CFGATLayer (masked graph-attention layer).

Math (per batch b):
  Q = x @ (W_q/sqrt(F)); K = x @ W_k; V = x @ W_v     # [N, F]
  S = Q @ K^T                                         # [N, N] (scale folded
  S -= BIG * (adj == 0)                               #  into W_q host-side)
  A = softmax(S, axis=-1); out = A @ V                # [N, F]

Distribution: batch dim (16) sharded over 8 NeuronCores, 2 batches per core.
adj is uploaded as adjC = BIG*(adj==0) in fp8e5 (1 byte, exact: BIG=28672 =
1.75*2^14), 4x less HBM traffic than the int32 original.

Per 128-row q-tile, the masked-softmax row-max pipeline is split per
1024-wide half to decouple PSUM residency (the v1..v3 attempts were either
DVE-bound or convoyed on psum-slot reuse):
  h0:  PE scores (f32r) -> psum; on even tiles DVE scalar_tensor_tensor
       computes t0 = S - adjC INTO SBUF in one 1x pass (additive mask +
       psum drain); on odd tiles (alt_drain=2) the mask runs on PE (ineg
       matmul accumulate) and ACT does the psum->t0 drain copy while DVE
       rowmaxes the psum in parallel -- this alternation splits the
       ~1.2us/tile drain between the DVE and ACT pacers 50/50.
  h1:  PE scores + PE mask-accumulate (psum += -1 * (I_fp8 @ adjC), an
       identity-stationary matmul streaming adjC); DVE rowmax of the psum.
  negm = -max(h0,h1 maxes) (DVE, negate=True).
  ACT  exp(. + bias=negm) -> bf16 e, h0 from SBUF t0, h1 from psum (the
       psum drain); per-partition bias AP fuses the subtract.
  DMA  XBAR dma_start_transpose of the group's e [P, GRP, N] -> eT
       [P, GRP*nkc, P] in ONE transfer (14ns per 16x128 tile); replaces
       v1's PE transpose passes. eT[:, j::nkc, :] is chunk j's [P, GW]
       moving slice for PV.
  PE   PV *flipped* (v6): o[q, Fa] += eT_chunk.T @ v_chunk per TILE --
       the eT chunk is the stationary operand and V (65 cols incl the
       ones-column denominator) streams, so PE streams 16*65 cols per
       tile instead of 16*512 per group and the output lands [q, F]
       directly: v4's [Fa, GW] oT psum, its ACT drain and the 4 PE
       re-transposes per group are all gone.  PV runs pv_dist groups
       behind the compute stream (one tile per pipeline slot,
       interleaved between tile halves) so eT is always ready.
  tail reciprocal of col F (DVE) + scale copy (ACT, per-partition scale
       AP) lag one pipeline slot behind their PV (tail_lag) so the
       strict-FIFO DVE/ACT queues never stall on an in-flight PV;
       grouped stores dispatch from the idle GPSIMD SWDGE queue.
adj loads and result stores dispatch from the idle GPSIMD SWDGE queue so
they never queue behind un-ready XBAR transposes on the SP HWDGE queue.

x is uploaded pre-transposed [f, n] (host transpose), removing the PE
setup transposes and their ACT psum drain.

Engine busy per core (TimelineSim): DVE ~98us and ACT ~100us
(co-pacers; tensor_reduce has no fast perf mode and GPSIMD tensor
ops/psum reads are rejected by this walrus build, so the full
2048-elem/row masked rowmax scan stays on DVE and exp on ACT -- the
h0 drain is the only fungible work and alt_drain splits it), DMA
~90us (57 XBAR + 33 HBM, serialized on the shared SDMA engines), PE
~75us; span 143us vs v4's 159us and v1's 204us.  The final group's PV runs per tile off per-tile XBARs
(last_tile_xbar) to shrink the end-of-kernel drain.

This compiler build accepts only one semaphore-wait command per instruction;
_split_excess_waits() legalizes the BIR by hoisting excess waits onto
EventSemaphore instructions (same engine => same sequencer order =>
identical semantics). The fused mask+max DVE ops (stock TENSOR_TENSOR_REDUCE
and custom InstCustomDveAnt tables) are rejected by this walrus build
(CoreV2GenImpl visitInstISA), hence the split STT + tensor_reduce pipeline.
"""

import sys

import numpy as np

sys.path.insert(0, "/opt/trn_rl_repo")

B, N, F = 16, 2048, 64

# pipeline-shape knobs (swept in sim; see sweep.py)
KCFG = dict(xbar_group=True, pv_dist=2, tail_lag=1, alt_drain=2, between=True,
            e_bufs=2, eT_bufs=3, adj_bufs=12, s_bufs=3, adjb=2, asym=False,
            defer_tail=False, spread_tail=False, last_tile_xbar=True,
            exp_split=False, flush_subtile=True)
NCORES = 8
NB = B // NCORES  # batches per core
P = 128  # partitions / q-tile rows
BIG = 28672.0  # exactly representable in fp8e5 (1.75 * 2^14)

_PATCHED = False


def _split_excess_waits(bir: bytes) -> bytes:
    """This compiler build only accepts one semaphore-wait command per
    instruction; hoist excess waits onto EventSemaphore instructions placed
    immediately before (same engine => same sequencer order => identical
    semantics)."""
    import orjson
    m = orjson.loads(bir)
    for fn in m["functions"]:
        for blk in fn["blocks"]:
            out = []
            for inst in blk["instructions"]:
                si = inst.get("sync_info")
                waits = (si or {}).get("on_wait") or []
                if len(waits) > 1:
                    for i, w in enumerate(waits[:-1]):
                        out.append({
                            "debug": inst.get("debug"),
                            "engine": inst["engine"],
                            "ins": [], "outs": [],
                            "name": f"{inst['name']}_w{i}",
                            "opcode": "EventSemaphore",
                            "sync_info": {"on_update": [], "on_wait": [w]},
                        })
                    si["on_wait"] = waits[-1:]
                out.append(inst)
            blk["instructions"] = out
    return orjson.dumps(m)


def _install_compile_patch():
    global _PATCHED
    if _PATCHED:
        return
    from concourse import bass_utils, bass2jax

    orig = bass_utils.compile_bir_kernel

    def patched(bir_json, tmpdir, neff_name="file.neff"):
        if isinstance(bir_json, str):
            bir_json = bir_json.encode()
        return orig(_split_excess_waits(bir_json), tmpdir, neff_name=neff_name)

    bass_utils.compile_bir_kernel = patched
    bass2jax.compile_bir_kernel = patched
    _PATCHED = True


def build_kernel(tc, out2, x2, adjc2, wq, wk, wv, nb, n, f):
    import concourse.bass as bass
    from concourse import mybir
    from concourse.masks import make_identity

    nc = tc.nc
    f32 = mybir.dt.float32
    f32r = mybir.dt.float32r
    bf16 = mybir.dt.bfloat16
    fp8 = mybir.dt.float8e5
    nqt = n // P          # q tiles per batch
    nkc = n // P          # key chunks (contraction chunks for PV)
    W = n // 2            # psum half width
    SW = 512              # matmul strip width
    GRP = 4 if nqt % 4 == 0 else 1   # q-tiles per PV group
    GW = GRP * P          # group width in q rows
    Fa = f + 1            # V augmented with ones column
    ADJB = KCFG.get("adjb", 4)  # q-tiles per adj DMA

    singles_cm = tc.tile_pool(name="singles", bufs=1)
    singles = singles_cm.__enter__()

    ident_f = singles.tile([P, P], f32)
    make_identity(nc, ident_f)
    ineg = singles.tile([P, P], fp8)
    nc.vector.tensor_scalar(
        out=ineg, in0=ident_f, scalar1=-1.0, scalar2=None,
        op0=mybir.AluOpType.mult,
    )

    wq_sb = singles.tile([f, f], f32)
    wk_sb = singles.tile([f, f], f32)
    wv_sb = singles.tile([f, f], f32)
    nc.sync.dma_start(out=wq_sb, in_=wq)
    nc.sync.dma_start(out=wk_sb, in_=wk)
    nc.sync.dma_start(out=wv_sb, in_=wv)
    wq_r = singles.tile([f, f], f32r)
    wk_r = singles.tile([f, f], f32r)
    wv_r = singles.tile([f, f], f32r)
    nc.vector.tensor_copy(wq_r, wq_sb)
    nc.vector.tensor_copy(wk_r, wk_sb)
    nc.vector.tensor_copy(wv_r, wv_sb)

    # persistent per-batch tensors
    xT1_sb = singles.tile([f, n], f32r)      # batch-1 x^T (late setup)
    qt_sb = singles.tile([f, nb, n], f32r)   # Q^T per batch (pre-scaled)
    kt_sb = singles.tile([f, nb, n], f32r)
    v_sb = singles.tile([P, nb, nkc, Fa], bf16)  # V (+ones col) by key chunk

    # main-loop SBUF pools allocated before the setup pools so their
    # addresses are disjoint from setup scratch
    adj_p_cm = tc.tile_pool(name="adj_p", bufs=KCFG["adj_bufs"])
    t_p_cm = tc.tile_pool(name="t_p", bufs=KCFG.get("t_bufs", 3))
    e_p_cm = tc.tile_pool(name="e_p", bufs=KCFG["e_bufs"])
    eT_p_cm = tc.tile_pool(name="eT_p", bufs=KCFG["eT_bufs"])
    small_cm = tc.tile_pool(name="small", bufs=KCFG.get("small_bufs", 8))
    osb_p_cm = tc.tile_pool(name="osb_p", bufs=KCFG.get("osb_bufs", 8))
    res_p_cm = tc.tile_pool(name="res_p", bufs=2)
    adj_p = adj_p_cm.__enter__()
    t_p = t_p_cm.__enter__()
    e_p = e_p_cm.__enter__()
    eT_p = eT_p_cm.__enter__()
    small = small_cm.__enter__()
    osb_p = osb_p_cm.__enter__()
    res_p = res_p_cm.__enter__()

    # ---------------- setup: QKV ----------------
    with tc.tile_pool(name="setup_ps", bufs=2, space="PSUM") as setup_ps, \
         tc.tile_pool(name="setup_sb", bufs=2) as setup_sb:
        n_setup = 1 if KCFG.get("late_setup") else nb
        if KCFG.get("late_setup"):
            nc.scalar.dma_start(out=xT1_sb, in_=x2[1])
        for b in range(n_setup):
            # x is uploaded pre-transposed [f, n]; read as f32r directly
            xT_sb = setup_sb.tile([f, n], f32r, tag="xT")
            nc.scalar.dma_start(out=xT_sb, in_=x2[b])

            # Q^T/K^T : [f, n] = W^T @ x^T
            qt_ps = setup_ps.tile([f, n], f32, tag="big")
            for j in range(n // SW):
                nc.tensor.matmul(
                    qt_ps[:, j * SW:(j + 1) * SW],
                    lhsT=wq_r,
                    rhs=xT_sb[:, j * SW:(j + 1) * SW],
                    start=True, stop=True,
                )
            if KCFG.get("qt128") and b == 0:
                # tile 0 only needs qt cols 0:128 -- drain them first so
                # the first score matmul isn't gated on the full copy
                nc.scalar.copy(qt_sb[:, b, 0:P], qt_ps[:, 0:P])
                nc.scalar.copy(qt_sb[:, b, P:n], qt_ps[:, P:n])
            elif KCFG.get("strip_drain") and b == 0:
                for j in range(n // SW):
                    nc.scalar.copy(qt_sb[:, b, j * SW:(j + 1) * SW],
                                   qt_ps[:, j * SW:(j + 1) * SW])
            elif KCFG.get("setup_dve"):
                nc.vector.tensor_copy(qt_sb[:, b, :], qt_ps)
            else:
                nc.scalar.copy(qt_sb[:, b, :], qt_ps)
            kt_ps = setup_ps.tile([f, n], f32, tag="big")
            for j in range(n // SW):
                nc.tensor.matmul(
                    kt_ps[:, j * SW:(j + 1) * SW],
                    lhsT=wk_r,
                    rhs=xT_sb[:, j * SW:(j + 1) * SW],
                    start=True, stop=True,
                )
            if (KCFG.get("qt128") or KCFG.get("strip_drain")) and b == 0:
                # h0 of tile 0 needs only the first kt half
                for j in range(2):
                    nc.scalar.copy(kt_sb[:, b, j * W:(j + 1) * W],
                                   kt_ps[:, j * W:(j + 1) * W])
            elif KCFG.get("setup_dve") == 2:
                nc.vector.tensor_copy(kt_sb[:, b, :], kt_ps)
            else:
                nc.scalar.copy(kt_sb[:, b, :], kt_ps)

            # V chunks: v[kchunk] = x[kchunk] @ W_v -> [128, f] (bf16 + ones)
            v_ps = setup_ps.tile([P, nkc, f], f32, tag="big")
            for t in range(nkc):
                nc.tensor.matmul(
                    v_ps[:, t, :], lhsT=xT_sb[:, t * P:(t + 1) * P],
                    rhs=wv_r, start=True, stop=True,
                )
            if KCFG.get("setup_dve"):
                nc.vector.tensor_copy(v_sb[:, b, :, 0:f], v_ps)
            else:
                nc.scalar.copy(v_sb[:, b, :, 0:f], v_ps)
        # ones column for the softmax denominator
        nc.vector.memset(v_sb[:, :, :, f:Fa], 1.0)

    # ---------------- main loop ----------------
    asym = KCFG.get("asym", False)
    with tc.tile_pool(name="s_ps", bufs=(2 if asym else KCFG["s_bufs"]),
                      space="PSUM") as s_ps_pool, \
         tc.tile_pool(name="s2_ps", bufs=3, space="PSUM") as s2_ps_pool, \
         tc.tile_pool(name="o_ps", bufs=KCFG.get("o_bufs", 2),
                      space="PSUM") as o_ps_pool:


        # PV (flipped) for one tile of a group: o[q, Fa] accumulates
        # eT_chunk.T @ v_chunk -- eT is the stationary operand, V (+ones
        # col, the softmax denominator) streams 65 cols per chunk.  Output
        # lands [q, Fa] directly: no [Fa, GW] oT psum and no PE
        # re-transpose tail.
        def emit_pv(o_ps, eT_fn, b0, g):
            for j in range(nkc):
                nc.tensor.matmul(
                    o_ps,
                    lhsT=eT_fn(g, j),
                    rhs=v_sb[:, b0, j, :],
                    start=(j == 0), stop=(j == nkc - 1),
                )

        # tail: with o_drain the o psum is copied to SBUF on ACT right
        # after the PV (frees the psum bank and decouples the recip from
        # the in-flight PV so it never blocks DVE's FIFO); the lagged
        # finish then runs r = 1/denominator and res = o * r on DVE from
        # SBUF (2x_2p mode, cheap); grouped store every GRP tiles from
        # the idle GPSIMD SWDGE queue.
        def emit_tail(o_t, b0, qi, res_sb, last=False):
            g = qi % GRP
            r1 = small.tile([P, 1], f32, tag="r1", name="r1")
            nc.vector.reciprocal(r1, o_t[:, f:Fa])
            if last or KCFG.get("o_drain"):
                nc.vector.tensor_scalar_mul(res_sb[:, g, :], o_t[:, 0:f], r1)
            else:
                nc.scalar.activation(
                    out=res_sb[:, g, :], in_=o_t[:, 0:f],
                    func=mybir.ActivationFunctionType.Copy,
                    scale=r1,
                )
            if g == GRP - 1:
                eng = nc.sync if last else nc.gpsimd
                eng.dma_start(
                    out=out2[b0, (qi - (GRP - 1)) * P:(qi + 1) * P, :]
                        .rearrange("(i p) f -> p i f", p=P),
                    in_=res_sb,
                )

        # late setup for batch 1, interleaved with batch 0's tiles
        # (strip-wise through the main psum pool)
        def late_setup(step):
            if step in (1, 2):
                w_r = wq_r if step == 1 else wk_r
                dst = qt_sb if step == 1 else kt_sb
                for hh in range(2):
                    ps = s_ps_pool.tile([f, W], f32, tag="s", name="qk_ps")
                    for j in range(W // SW):
                        nc.tensor.matmul(
                            ps[:, j * SW:(j + 1) * SW],
                            lhsT=w_r,
                            rhs=xT1_sb[:, hh * W + j * SW:hh * W + (j + 1) * SW],
                            start=True, stop=True,
                        )
                    nc.scalar.copy(dst[:, 1, hh * W:(hh + 1) * W], ps)
            elif step == 3:
                v_ps = s_ps_pool.tile([P, nkc, f], f32, tag="s", name="v_ps")
                for t in range(nkc):
                    nc.tensor.matmul(
                        v_ps[:, t, :], lhsT=xT1_sb[:, t * P:(t + 1) * P],
                        rhs=wv_r, start=True, stop=True,
                    )
                nc.scalar.copy(v_sb[:, 1, :, 0:f], v_ps)

        # Software pipeline: XBAR transposes (per tile or per group) fill
        # eT; the group's PV runs pv_dist groups later, one slice per tile,
        # so the eT input is pipeline-distant. adj loads and result stores
        # dispatch from the idle GPSIMD queue (SWDGE) so they never queue
        # behind un-ready XBAR transposes on the SP HWDGE queue.
        xg = KCFG["xbar_group"]
        # groups awaiting PV:
        # [eT_fn, b, last_qi, res_sb or None, last_grp, next_tile]
        pv_q = []
        tails = []  # tiles awaiting recip/scale/store, lagged so the
                    # DVE/ACT FIFO queues never stall on an in-flight PV

        def do_tail():
            o_ps, b0, qi2, res_sb, last2 = tails.pop(0)
            emit_tail(o_ps, b0, qi2, res_sb, last=last2)

        def pipeline_work(force=False):
            if pv_q and (force or len(pv_q) >= KCFG["pv_dist"]):
                ent = pv_q[0]
                g = ent[5]
                ent[5] += 1
                if ent[3] is None:
                    ent[3] = res_p.tile([P, GRP, f], f32, tag="res",
                                        name="res_sb")
                o_ps = o_ps_pool.tile([P, Fa], f32, tag="o", name="o_ps")
                emit_pv(o_ps, ent[0], ent[1], g)
                if KCFG.get("o_drain"):
                    o_sb = osb_p.tile([P, Fa], f32, tag="osb", name="o_sb")
                    nc.scalar.copy(o_sb, o_ps)
                    o_t = o_sb
                else:
                    o_t = o_ps
                tails.append((o_t, ent[1], ent[2] - (GRP - 1) + g, ent[3],
                              ent[4]))
                if g == GRP - 1:
                    pv_q.pop(0)
            while len(tails) > KCFG.get("tail_lag", 1):
                do_tail()

        for b in range(nb):
            e_grp = None
            eT_sb = None
            eTg = None
            adj_t = None
            for qi in range(nqt):
                g = qi % GRP
                if g == 0:
                    if xg:
                        e_grp = e_p.tile([P, GRP, n], bf16, tag="e")
                    else:
                        eT_sb = eT_p.tile([P, nkc, GW], bf16, tag="eT")

                if qi % ADJB == 0:
                    adj_t = adj_p.tile([P, ADJB, n], fp8, tag="adj")
                    nc.gpsimd.dma_start(
                        out=adj_t,
                        in_=adjc2[b, qi * P:(qi + ADJB) * P, :].rearrange(
                            "(t p) k -> p t k", p=P),
                    )
                adj_v = adj_t[:, qi % ADJB, :]

                if asym:
                    # h0 = 512 cols via DVE additive-STT (frees psum fast);
                    # h1 = 1536 cols PE-masked in [1024]+[512] psum tiles.
                    m3 = small.tile([P, 3], f32, tag="m3")
                    t0_sb = t_p.tile([P, SW], f32, tag="t0")
                    s_halves = []
                    s0 = s2_ps_pool.tile([P, SW], f32, tag="s2", name="s0")
                    nc.tensor.matmul(
                        s0, lhsT=qt_sb[:, b, qi * P:(qi + 1) * P],
                        rhs=kt_sb[:, b, 0:SW], start=True, stop=True,
                    )
                    nc.vector.scalar_tensor_tensor(
                        out=t0_sb, in0=s0, scalar=1.0,
                        in1=adj_v[:, 0:SW],
                        op0=mybir.AluOpType.mult,
                        op1=mybir.AluOpType.subtract,
                    )
                    nc.vector.tensor_reduce(
                        out=m3[:, 0:1], in_=t0_sb,
                        axis=mybir.AxisListType.X, op=mybir.AluOpType.max,
                    )
                    if KCFG["between"]:
                        pipeline_work()
                    for part, (lo, wid) in enumerate(
                            ((SW, W), (SW + W, SW))):
                        sp = (s_ps_pool if wid == W else s2_ps_pool).tile(
                            [P, wid], f32, tag="s" if wid == W else "s2",
                            name="s1")
                        s_halves.append(sp)
                        for j in range(wid // SW):
                            nc.tensor.matmul(
                                sp[:, j * SW:(j + 1) * SW],
                                lhsT=qt_sb[:, b, qi * P:(qi + 1) * P],
                                rhs=kt_sb[:, b, lo + j * SW:lo + (j + 1) * SW],
                                start=True, stop=True,
                            )
                        for j in range(wid // SW):
                            nc.tensor.matmul(
                                sp[:, j * SW:(j + 1) * SW],
                                lhsT=ineg,
                                rhs=adj_v[:, lo + j * SW:lo + (j + 1) * SW],
                                start=False, stop=True,
                            )
                        nc.vector.tensor_reduce(
                            out=m3[:, 1 + part:2 + part], in_=sp,
                            axis=mybir.AxisListType.X, op=mybir.AluOpType.max,
                        )
                else:
                    m3 = small.tile(
                        [P, 3 if KCFG.get("h1_strips") else 2], f32, tag="m3")
                    import contextlib
                    hp = (tc.high_priority(KCFG["dve_prio"])
                          if KCFG.get("dve_prio") else contextlib.nullcontext())
                    t0_sb = t_p.tile([P, W], f32, tag="t0")
                    s_halves = []
                    for h in range(2):
                        if h == 1 and KCFG["between"] and not KCFG.get("pw_late"):
                            pipeline_work()
                        s_ps = s_ps_pool.tile([P, W], f32, tag="s")
                        s_halves.append(s_ps)
                        for j in range(W // SW):
                            nc.tensor.matmul(
                                s_ps[:, j * SW:(j + 1) * SW],
                                lhsT=qt_sb[:, b, qi * P:(qi + 1) * P],
                                rhs=kt_sb[:, b,
                                          h * W + j * SW:h * W + (j + 1) * SW],
                                start=True, stop=True,
                            )
                        # per-tile h0 drain mode: 's' = DVE STT (mask +
                        # drain in one pass), 'a' = PE-mask + ACT drain
                        # copy, 'n' = PE-mask + no drain (exp reads psum)
                        pat = KCFG.get("drain_pat")
                        if pat:
                            dmode = pat[(b * nqt + qi) % len(pat)]
                        else:
                            alt = KCFG.get("alt_drain", 0)
                            dmode = ("a" if alt and qi % alt == alt - 1
                                     else "s")
                        tile_nd = dmode == "n"
                        if h == 0 and dmode in ("a", "n"):
                            # DVE/ACT load-balance: every alt-th tile masks
                            # h0 on PE (ineg matmul accumulate) and drains
                            # the psum to t0 via an ACT copy; DVE's rowmax
                            # reads the psum in parallel with the copy.
                            # Shifts ~1.2us/tile of STT work off the DVE
                            # pacer onto ACT's slack.
                            for j in range(W // SW):
                                nc.tensor.matmul(
                                    s_ps[:, j * SW:(j + 1) * SW],
                                    lhsT=ineg,
                                    rhs=adj_v[:, j * SW:(j + 1) * SW],
                                    start=False, stop=True,
                                )
                            if dmode == "a":
                                nc.scalar.copy(t0_sb, s_ps)
                            nc.vector.tensor_reduce(
                                out=m3[:, 0:1], in_=s_ps,
                                axis=mybir.AxisListType.X, op=mybir.AluOpType.max,
                            )
                        elif h == 0 and KCFG.get("mask_both"):
                            # mask h0 on PE too; DVE reduces the psum
                            # directly (no STT drain -- psum freed by exp)
                            for j in range(W // SW):
                                nc.tensor.matmul(
                                    s_ps[:, j * SW:(j + 1) * SW],
                                    lhsT=ineg,
                                    rhs=adj_v[:, j * SW:(j + 1) * SW],
                                    start=False, stop=True,
                                )
                            nc.vector.tensor_reduce(
                                out=m3[:, 0:1], in_=s_ps,
                                axis=mybir.AxisListType.X, op=mybir.AluOpType.max,
                            )
                        elif h == 0:
                            # additive mask + psum drain on DVE in one pass:
                            # t0 = S - adjC  (adjC = BIG where masked, 0 else).
                            # Frees the psum slot without touching ACT.
                            nc.vector.scalar_tensor_tensor(
                                out=t0_sb, in0=s_ps, scalar=1.0,
                                in1=adj_v[:, 0:W],
                                op0=mybir.AluOpType.mult,
                                op1=mybir.AluOpType.subtract,
                            )
                            nc.vector.tensor_reduce(
                                out=m3[:, 0:1], in_=t0_sb,
                                axis=mybir.AxisListType.X, op=mybir.AluOpType.max,
                            )
                        else:
                            # mask accumulate: s += -1 * (I @ adjC)
                            for j in range(W // SW):
                                nc.tensor.matmul(
                                    s_ps[:, j * SW:(j + 1) * SW],
                                    lhsT=ineg,
                                    rhs=adj_v[:, W + j * SW:W + (j + 1) * SW],
                                    start=False, stop=True,
                                )
                            nc.vector.tensor_reduce(
                                out=m3[:, 1:2], in_=s_ps,
                                axis=mybir.AxisListType.X,
                                op=mybir.AluOpType.max,
                            )
                # negm = -rowmax over the strip maxes; the negm->exp
                # chain is the tile-rate critical path, so optionally
                # schedule it ahead of competing same-engine work
                import contextlib
                ep_cm = (tc.high_priority(KCFG["exp_prio"])
                         if KCFG.get("exp_prio") else contextlib.nullcontext())
                ep_cm.__enter__()
                negm = small.tile([P, 1], f32, tag="negm")
                nc.vector.tensor_reduce(
                    out=negm, in_=m3, axis=mybir.AxisListType.X,
                    op=mybir.AluOpType.max, negate=True,
                )

                # e = exp(s - m) bf16, q-major (fused subtract via bias);
                # h0 reads SBUF t0, h1 reads (and thereby frees) psum
                e_dst = e_grp[:, g, :] if xg else e_p.tile([P, n], bf16, tag="e", name="e_sb")
                if asym:
                    for src, lo, wid in ((t0_sb, 0, SW),
                                         (s_halves[0], SW, W),
                                         (s_halves[1], SW + W, SW)):
                        nc.scalar.activation(
                            out=e_dst[:, lo:lo + wid], in_=src,
                            func=mybir.ActivationFunctionType.Exp,
                            bias=negm, scale=1.0,
                        )
                elif (KCFG.get("exp_h1_first")
                      or (KCFG.get("h1f_a") and dmode == "a")):
                    # free the scarce psum slot first, then the t-slot
                    nc.scalar.activation(
                        out=e_dst[:, W:n], in_=s_halves[1],
                        func=mybir.ActivationFunctionType.Exp,
                        bias=negm, scale=1.0,
                    )
                    nc.scalar.activation(
                        out=e_dst[:, 0:W], in_=t0_sb,
                        func=mybir.ActivationFunctionType.Exp,
                        bias=negm, scale=1.0,
                    )
                else:
                    nc.scalar.activation(
                        out=e_dst[:, 0:W],
                        in_=(s_halves[0]
                             if KCFG.get("mask_both") or tile_nd else t0_sb),
                        func=mybir.ActivationFunctionType.Exp,
                        bias=negm, scale=1.0,
                    )
                if asym or KCFG.get("exp_h1_first"):
                    pass
                elif KCFG.get("exp_split"):
                    for j in range(W // SW):
                        nc.scalar.activation(
                            out=e_dst[:, W + j * SW:W + (j + 1) * SW],
                            in_=s_halves[1][:, j * SW:(j + 1) * SW],
                            func=mybir.ActivationFunctionType.Exp,
                            bias=negm, scale=1.0,
                        )
                else:
                    nc.scalar.activation(
                        out=e_dst[:, W:n], in_=s_halves[1],
                        func=mybir.ActivationFunctionType.Exp,
                        bias=negm, scale=1.0,
                    )

                ep_cm.__exit__(None, None, None)
                if KCFG.get("pw2"):
                    pipeline_work()

                if not xg:
                    # XBAR transpose: eT[p, j, g*128+q] = e[q, j*128+p]
                    nc.sync.dma_start_transpose(
                        out=eT_sb[:, :, g * P:(g + 1) * P], in_=e_dst,
                    )

                if not KCFG["between"] or KCFG.get("pw_late"):
                    pipeline_work()

                last_grp = (b == nb - 1) and (qi >= nqt - GRP)
                if xg == "tile" or (xg and KCFG.get("last_tile_xbar") and last_grp):
                    # per-tile XBAR into a contiguous slice of the group
                    # tile: eT[p, g*nkc+j, q] = e_grp[q, g, j*128+p]
                    if g == 0:
                        eTg = eT_p.tile([P, GRP * nkc, P], bf16, tag="eT")
                    if qi >= nqt - KCFG.get("chunk_xbar_last", 0):
                        # final tile(s): one XBAR per 128-col k-chunk so
                        # each PV chunk starts as soon as its slice lands
                        # (and the h0-cols chunks start while the h1 exp
                        # is still running), shrinking the teardown tail
                        for j in range(nkc):
                            nc.sync.dma_start_transpose(
                                out=eTg[:, g * nkc + j, :],
                                in_=e_grp[:, g, j * P:(j + 1) * P])
                    else:
                        nc.sync.dma_start_transpose(
                            out=eTg[:, g * nkc:(g + 1) * nkc, :],
                            in_=e_grp[:, g, :])
                elif xg == "half":
                    # two XBARs per group: contiguous 2-tile slices
                    if g == 1:
                        eTg = eT_p.tile([P, GRP * nkc, P], bf16, tag="eT")
                    if g % 2 == 1:
                        nc.sync.dma_start_transpose(
                            out=eTg[:, (g - 1) * nkc:(g + 1) * nkc, :],
                            in_=e_grp[:, g - 1:g + 1, :])
                elif xg == "half":
                    pass
                elif xg and g == GRP - 1 and not (KCFG.get("last_tile_xbar") and last_grp):
                    # one XBAR for the whole group:
                    # eT[p, t*nkc+j, q] = e_grp[q, t, j*128+p]
                    eTg = eT_p.tile([P, GRP * nkc, P], bf16, tag="eT")
                    nc.sync.dma_start_transpose(out=eTg, in_=e_grp)

                ls0 = KCFG.get("late_setup") or 99
                if b == 0 and ls0 <= qi <= ls0 + 2:
                    late_setup(qi - ls0 + 1)

                if g == GRP - 1:
                    if xg:
                        eT_fn = (lambda eT0: lambda t, j:
                                 eT0[:, t * nkc + j, :])(eTg)
                    else:
                        eT_fn = (lambda eT0: lambda t, j:
                                 eT0[:, j, t * P:(t + 1) * P])(eT_sb)
                    last_grp = (b == nb - 1) and (qi >= nqt - GRP)
                    pv_q.append([eT_fn, b, qi, None, last_grp, 0])

                # end-game catch-up: during the last groups, pump the PV
                # pipeline an extra slot per tile so the backlog drains
                # before the loop ends (shrinks the serial teardown tail)
                cz = KCFG.get("catchup", 0)
                if cz and b == nb - 1 and qi >= nqt - cz:
                    pipeline_work(force=True)

        # flush the remaining groups' PV + tails (per tile, so each chain
        # starts as soon as that tile's XBAR lands)
        while pv_q:
            pipeline_work(force=True)
        while tails:
            do_tail()
    for cm in (res_p_cm, osb_p_cm, small_cm, eT_p_cm, e_p_cm, t_p_cm, adj_p_cm):
        cm.__exit__(None, None, None)
    singles_cm.__exit__(None, None, None)


def build_bass(nb=NB, n=N, f=F, num_devices=NCORES):
    import concourse.bass as bass
    import concourse.tile as tile
    from concourse import mybir

    nc = bass.Bass(
        "TRN2", target_bir_lowering=False, debug=False, num_devices=num_devices
    )
    x2 = nc.dram_tensor("x2", [nb, f, n], mybir.dt.float32r,
                        kind="ExternalInput").ap()
    adjc2 = nc.dram_tensor("adjc2", [nb, n, n], mybir.dt.float8e5,
                           kind="ExternalInput").ap()
    wq = nc.dram_tensor("wq", [f, f], mybir.dt.float32, kind="ExternalInput").ap()
    wk = nc.dram_tensor("wk", [f, f], mybir.dt.float32, kind="ExternalInput").ap()
    wv = nc.dram_tensor("wv", [f, f], mybir.dt.float32, kind="ExternalInput").ap()
    out2 = nc.dram_tensor("out2", [nb, n, f], mybir.dt.float32,
                          kind="ExternalOutput").ap()
    with tile.TileContext(nc) as tc:
        build_kernel(tc, out2, x2, adjc2, wq, wk, wv, nb=nb, n=n, f=f)
    return nc


_cached_nc = None


def kernel(x, adj, W_q, W_k, W_v, _trace=False):
    global _cached_nc
    _install_compile_patch()
    import ml_dtypes
    from concourse import bass_utils

    if _cached_nc is None:
        _cached_nc = build_bass()
    nc = _cached_nc

    x = np.ascontiguousarray(
        np.asarray(x, dtype=np.float32).transpose(0, 2, 1))
    adj = np.asarray(adj)
    # adjC = BIG where masked (adj == 0), 0 else, as 1-byte fp8e5
    adjc = np.ascontiguousarray(
        ((adj == 0).astype(np.float32) * BIG).astype(ml_dtypes.float8_e5m2))
    scale = 1.0 / np.sqrt(np.float32(F))
    wq = np.ascontiguousarray(np.asarray(W_q, dtype=np.float32) * scale)
    wk = np.ascontiguousarray(np.asarray(W_k, dtype=np.float32))
    wv = np.ascontiguousarray(np.asarray(W_v, dtype=np.float32))

    in_maps = []
    for c in range(NCORES):
        in_maps.append({
            "x2": x[c * NB:(c + 1) * NB],
            "adjc2": adjc[c * NB:(c + 1) * NB],
            "wq": wq, "wk": wk, "wv": wv,
        })
    res = bass_utils.run_bass_kernel_spmd(
        nc, in_maps, core_ids=list(range(NCORES)), trace=_trace,
    )
    out = np.concatenate([r["out2"] for r in res.results], axis=0)
    if _trace:
        kernel._last_results = res
    return out.reshape(B, N, F)

